# revision 48
# baseline (speedup 1.0000x reference)
"""Trainium2 Bass kernel for nn_Attention (B=4, N=2048, 12 heads, d=64).

Per-core work (core = (batch b, head-group hg)): 6 heads of attention over
N=2048, plus its slice of the qkv/out projections. Host splits w_qkv/w_proj
by head group, pre-scales q, and sums the two half-projections per batch
(plus bias) at the end. No collectives — every core's inputs are host-sliced.

Design notes (vs the V2 baseline, 355us HW / 258us CoreSim):
- The softmax exp stream on the ACT engine is the binding resource
  (~1 el/cycle/lane @1.2GHz + ~350cyc/instr overhead): everything is
  organized to keep it gapless. Score chunks [128kv x 1024q] ping-pong
  through 2x2 PSUM banks; exp instructions span the full chunk.
- A third of the h2=1 exp chunks run on DVE instead, via a Schraudolph
  fast-exp (int16(x*2^7/ln2 + 127*128-7) bitcast to bf16, ~1.8% rms);
  rebalances ACT vs the otherwise lightly-loaded DVE. Adds ~2e-3 rel err.
- AV runs in [q, d] orientation (lhsT = exp-score tile, rhs = v plus a
  ones-column for the softmax denominator), 65-wide streams: 2x fewer PE
  cycles than the [d, q] form under the cost model, wash on HW. The
  normalize is then a per-partition tensor_scalar; a PE transpose restores
  [d, q] for the projection lhsT.
- Minimal lead-in (2 qk groups on a packed lead DMA), remaining qkv/proj
  work trickles through the chunk steps as paced fillers; slot finishes are
  two-phase (AV+divide at next slot's step 0, transposes at step 1) so PE
  never waits on DVE; last strip's projections drain inline per q-tile.
- PE p-state warmup on the identity tile during the initial DMA wait.
"""

import sys

if "/opt/trn_rl_repo" not in sys.path:
    sys.path.insert(0, "/opt/trn_rl_repo")

import numpy as np
import ml_dtypes

import concourse.bacc as bacc
import concourse.mybir as mybir
import concourse.tile as tile
from concourse.masks import make_identity

FP32 = mybir.dt.float32
BF16 = mybir.dt.bfloat16
AF = mybir.ActivationFunctionType

DIM = 768
HEAD_DIM = 64
SCALE = HEAD_DIM ** -0.5
B, N = 4, 2048
HG = 6
CC = DIM // 128
PAIRS = HG // 2
S = N // 512
J = N // 128
CH = 2                      # kv blocks per score chunk
NCHUNK = J // CH            # 8 chunks per (head, strip)

# Schraudolph-style fast exp on DVE: bf16(x) ~ bitcast_bf16(int16(x*A + B))
# with A = 2^7/ln2 and B = 127*2^7 - C. C tuned numerically for min RMS
# relative error (~1.8%, max ~4%) under truncation. A third of the h2=1
# exp chunks run on DVE to offload the saturated ACT engine.
FAST_EXP_A = 128.0 / float(np.log(2.0))
FAST_EXP_B = 16256.0 - 7.0

_CACHED = {}


def build_core_program(reps=0, ablate=()):
    nc = bacc.Bacc("TRN2", debug=False, target_bir_lowering=False, num_devices=1)

    xt_d = nc.dram_tensor("xt", [DIM, N], BF16, kind="ExternalInput")
    wqk_d = nc.dram_tensor("wqk", [DIM, DIM], BF16, kind="ExternalInput")
    wv_d = nc.dram_tensor("wv", [DIM, HG * 64], BF16, kind="ExternalInput")
    wp_d = nc.dram_tensor("wp", [HG * 64, DIM], BF16, kind="ExternalInput")
    y_d = nc.dram_tensor("y", [N, DIM], FP32, kind="ExternalOutput")

    with tile.TileContext(nc) as tc:
        with (
            tc.tile_pool(name="persist", bufs=1) as persist,
            tc.tile_pool(name="exps", bufs=12) as exps_pool,
            tc.tile_pool(name="attnt", bufs=5) as attnt_pool,
            tc.tile_pool(name="small", bufs=6) as small_pool,
            tc.tile_pool(name="ysb", bufs=3) as y_pool,
            tc.tile_pool(name="ps_score", bufs=2, space="PSUM") as ps_score,
            tc.tile_pool(name="ps_av", bufs=2, space="PSUM") as ps_av,
            tc.tile_pool(name="ps_misc", bufs=2, space="PSUM") as ps_misc,
        ):
            xT = persist.tile([128, CC, N], BF16)
            wqk = persist.tile([128, CC, DIM], BF16)
            wv = persist.tile([128, CC, HG * 64], BF16)
            wp = persist.tile([128, PAIRS, DIM], BF16)
            qkT = persist.tile([128, CC, N], BF16)
            v = persist.tile([128, J, HG, 65], BF16)
            ident = persist.tile([128, 128], BF16)

            # wqk on SP and xT on Pool so the two issue streams run in
            # parallel (DMA issue is ~0.6us each and gates the lead-in);
            # wv/wp follow once the critical tensors are queued.
            xt_r = xt_d.ap().rearrange("(o p) n -> p o n", p=128)
            wqk_r = wqk_d.ap().rearrange("(o p) n -> p o n", p=128)
            wv_r = wv_d.ap().rearrange("(o p) n -> p o n", p=128)
            # The lead-in qk groups (pair 0, strip 0) only need wqk cols
            # {0:128, 384:512} and x strip 0: those land first as two packed
            # transfers (SP- and Pool-issued, in parallel); remainders and
            # wv/wp follow. Writers are disjoint so the lead-in reads never
            # wait on remainder DMAs.
            nc.sync.dma_start(out=wqk[:, :, 0:128], in_=wqk_r[:, :, 0:128])
            nc.scalar.dma_start(
                out=xT[:, 0:3, 0:512], in_=xt_r[:, 0:3, 0:512]
            )
            nc.scalar.dma_start(
                out=xT[:, 3:6, 0:512], in_=xt_r[:, 3:6, 0:512]
            )
            nc.sync.dma_start(
                out=wqk[:, :, 384:512], in_=wqk_r[:, :, 384:512]
            )
            for cc in range(CC):
                nc.sync.dma_start(out=wv[:, cc], in_=wv_r[:, cc])
            for cc in range(CC):
                nc.sync.dma_start(
                    out=xT[:, cc, 512:N], in_=xt_r[:, cc, 512:N]
                )
            nc.sync.dma_start(
                out=wqk[:, :, 128:384], in_=wqk_r[:, :, 128:384]
            )
            nc.sync.dma_start(
                out=wqk[:, :, 512:DIM], in_=wqk_r[:, :, 512:DIM]
            )
            nc.sync.dma_start(out=wp, in_=wp_d.ap().rearrange("(o p) n -> p o n", p=128))
            make_identity(nc, ident)
            # only the denominator ones-column needs initializing; the d
            # columns are fully written by v_groups
            nc.vector.memset(v[:, :, :, 64:65], 1.0)

            # warm the PE p-state during the DMA wait: a run of no-dep
            # matmuls on the identity tile carries the engine through the
            # slow ramp so the lead-in runs at full clock.
            # warmup lives in the AV pool: its first real allocation isn't
            # written until ~12us in, while a misc-pool slot would make the
            # second lead-in qk group wait for the whole warmup run
            warm = ps_av.tile([128, 4, 128], FP32, tag="av", name="warm")
            for _ in range(42):
                nc.tensor.matmul(warm[:, 0, 0:128], ident, ident,
                                 start=True, stop=True)

            # ---------- psum-group building blocks ----------

            def qk_group(ot, s):
                ps = ps_misc.tile([128, 512], FP32, tag="m")
                for cc in range(CC):
                    nc.tensor.matmul(
                        ps,
                        wqk[:, cc, ot * 128 : ot * 128 + 128],
                        xT[:, cc, s * 512 : s * 512 + 512],
                        start=(cc == 0), stop=(cc == CC - 1),
                    )
                nc.vector.tensor_copy(out=qkT[:, ot, s * 512 : s * 512 + 512], in_=ps)

            def v_group(p, nt):
                psv = ps_misc.tile([128, 512], FP32, tag="m")
                for cc in range(CC):
                    nc.tensor.matmul(
                        psv[:, 0:128],
                        xT[:, cc, nt * 128 : nt * 128 + 128],
                        wv[:, cc, p * 128 : p * 128 + 128],
                        start=(cc == 0), stop=(cc == CC - 1),
                    )
                for h2 in range(2):
                    nc.vector.tensor_copy(
                        out=v[:, nt, 2 * p + h2, 0:64],
                        in_=psv[:, h2 * 64 : h2 * 64 + 64],
                    )

            def proj_group(s, attnT, nt, og, ow, ysb):
                psy = ps_misc.tile([128, 512], FP32, tag="m")
                for cc in range(PAIRS):
                    nc.tensor.matmul(
                        psy[:, 0:ow],
                        attnT[:, cc, nt * 128 : nt * 128 + 128],
                        wp[:, cc, og : og + ow],
                        start=(cc == 0), stop=(cc == PAIRS - 1),
                    )
                nc.vector.tensor_copy(out=ysb[:, og : og + ow], in_=psy[:, 0:ow])
                # ship each column half as soon as it's ready so the final
                # tile's DMA only covers 256 columns
                row = s * 512 + nt * 128
                nc.sync.dma_start(
                    out=y_d.ap()[row : row + 128, og : og + ow],
                    in_=ysb[:, og : og + ow],
                )

            def qkv_pair_fillers(p):
                out = []
                for ot in (p, PAIRS + p):
                    for s in range(S):
                        out.append(lambda ot=ot, s=s: qk_group(ot, s))
                for nt in range(J):
                    out.append(lambda nt=nt: v_group(p, nt))
                return out

            def proj_fillers(s, attnT):
                out = []
                for nt in range(4):
                    ysb = y_pool.tile([128, DIM], FP32, tag="y")
                    for og, ow in ((0, 512), (512, 256)):
                        out.append(
                            lambda nt=nt, og=og, ow=ow, ysb=ysb:
                                proj_group(s, attnT, nt, og, ow, ysb)
                        )
                return out

            def body():
                # minimal lead-in: q strip 0 and k strip 0 of pair 0 only
                qk_group(0, 0)
                qk_group(PAIRS, 0)

                # the other 6 lead-in groups, forced into slot (0,0) steps
                # just before the score chunks that need them:
                # k strip s' is read by chunks 2s'..2s'+1; q strip s is read
                # from slot (0,s) on.
                forced = {
                    1: [lambda: qk_group(PAIRS, 1)],
                    2: [lambda: qk_group(0, 1)],
                    3: [lambda: qk_group(PAIRS, 2)],
                    4: [lambda: qk_group(0, 2)],
                    5: [lambda: qk_group(PAIRS, 3)],
                    6: [lambda: qk_group(0, 3)],
                }

                filler = []
                attnT_tiles = {}
                gchunk = [0]  # global chunk counter for fast-exp selection
                slots = [(hp, s) for hp in range(PAIRS) for s in range(S)]
                # filler pop opportunities remaining (every step of each
                # non-first slot)
                pop_steps_left = (len(slots) - 1) * NCHUNK
                prev_finish = None

                for slot_idx, (hp, s) in enumerate(slots):
                    first_slot = slot_idx == 0
                    if s == 0 and hp + 1 < PAIRS:
                        filler.extend(qkv_pair_fillers(hp + 1))
                    if hp == 0:
                        at = attnt_pool.tile(
                            [128, PAIRS, 512], BF16, tag="attnT",
                            name=f"attnT{s}",
                        )
                        attnT_tiles[s] = at
                    attnT = attnT_tiles[s]

                    pav = {}
                    for h2 in range(2):
                        pav[h2] = ps_av.tile(
                            [128, 4, 128], FP32, tag="av", name=f"pav{h2}"
                        )

                    expS = {0: [None] * NCHUNK, 1: [None] * NCHUNK}

                    def emit_qkt_exp(c, s=s, hp=hp, expS=expS):
                        g = gchunk[0]
                        gchunk[0] += 1
                        # which h2 (if any) of this chunk runs fast-exp on DVE
                        fast_h2 = ((g // 3) % 2) if g % 3 == 1 else -1
                        for h2, base in ((0, 0), (1, 64)):
                            pss = ps_score.tile(
                                [128, 512 * CH], FP32, tag="sc"
                            )
                            for jj in range(CH):
                                j = c * CH + jj
                                if "qkt" in ablate:
                                    continue
                                nc.tensor.matmul(
                                    pss[:, jj * 512 : jj * 512 + 512],
                                    qkT[base : base + 64, PAIRS + hp,
                                        j * 128 : j * 128 + 128],
                                    qkT[base : base + 64, hp,
                                        s * 512 : s * 512 + 512],
                                    start=True, stop=True,
                                    tile_position=(base, 0),
                                )
                            et = exps_pool.tile(
                                [128, 512 * CH], BF16, tag="e"
                            )
                            if "exp" in ablate:
                                nc.vector.memset(et[:, 0:4], 1.0)
                            elif h2 == fast_h2:
                                nc.vector.tensor_scalar(
                                    et.bitcast(mybir.dt.int16),
                                    pss,
                                    FAST_EXP_A,
                                    FAST_EXP_B,
                                    mybir.AluOpType.mult,
                                    mybir.AluOpType.add,
                                )
                            else:
                                nc.scalar.activation(
                                    out=et, in_=pss, func=AF.Exp,
                                )
                            expS[h2][c] = et

                    def emit_av(c, hp=hp, pav=pav, expS=expS):
                        # AV in [q, d] orientation: out[128 q-tile, 65] per
                        # (head, q-tile), accumulated over the 16 kv blocks.
                        # lhsT = exp-score slice [128 kv, 128 q]; rhs =
                        # v[kv, d + ones-col] gives the softmax denominator
                        # in column 64.
                        if "av" in ablate:
                            return
                        for h2 in range(2):
                            h = 2 * hp + h2
                            et = expS[h2][c]
                            for jj in range(CH):
                                j = c * CH + jj
                                for qt in range(4):
                                    # start/stop once per pav BANK: start
                                    # marks the whole 2KB zero-region
                                    # pending-zero, so the other qt groups'
                                    # first writes land on zeroed bytes.
                                    nc.tensor.matmul(
                                        pav[h2][:, qt, 0:65],
                                        et[:, jj * 512 + qt * 128
                                           : jj * 512 + qt * 128 + 128],
                                        v[:, j, h, :],
                                        start=(j == 0 and qt == 0),
                                        stop=(j == J - 1 and qt == 3),
                                        skip_group_check=(not (j == 0 and qt == 0)
                                                          and not (j == J - 1 and qt == 3)),
                                    )

                    last_slot = slot_idx == len(slots) - 1

                    def make_finish(hp=hp, s=s, pav=pav, expS=expS,
                                    attnT=attnT, first_slot=first_slot,
                                    last_slot=last_slot, emit_av=emit_av):
                        # two-phase finish: phase 1 (next slot's step 0)
                        # emits the last AV group + the DVE normalize chain
                        # (divide rows by the ones-column denominator, a
                        # per-partition scalar); phase 2 (next slot's step 1)
                        # emits the PE transposes back to [d, q] layout, by
                        # which time DVE has produced the normalized tiles.
                        prs = []

                        def emit_div(qt):
                            pr = small_pool.tile([128, 128], BF16, tag="p")
                            prs.append(pr)
                            for h2 in range(2):
                                rc = small_pool.tile([128, 1], FP32, tag="r")
                                if "av" in ablate:
                                    nc.vector.memset(rc, 1.0)
                                else:
                                    nc.vector.reciprocal(
                                        out=rc, in_=pav[h2][:, qt, 64:65]
                                    )
                                nc.vector.tensor_scalar(
                                    pr[:, h2 * 64 : h2 * 64 + 64],
                                    pav[h2][:, qt, 0:64],
                                    rc,
                                    None,
                                    mybir.AluOpType.mult,
                                )

                        def emit_transpose(qt):
                            pt = ps_misc.tile([128, 128], BF16, tag="m")
                            nc.tensor.transpose(pt, prs[qt], ident)
                            nc.vector.tensor_copy(
                                out=attnT[:, hp, qt * 128 : qt * 128 + 128],
                                in_=pt,
                            )

                        def finish_av_div():
                            emit_av(NCHUNK - 1)
                            if not last_slot:
                                for qt in range(4):
                                    emit_div(qt)

                        def finish_transpose():
                            if last_slot:
                                # drain: interleave per q-tile so each
                                # projection only waits on its own tile's
                                # divide/transpose/copyback chain
                                last_projs = proj_fillers(
                                    s, attnT_tiles.pop(s)
                                )
                                for qt in range(4):
                                    emit_div(qt)
                                    emit_transpose(qt)
                                    last_projs.pop(0)()
                                    last_projs.pop(0)()
                                return
                            for qt in range(4):
                                emit_transpose(qt)
                            if hp == PAIRS - 1:
                                filler.extend(
                                    proj_fillers(s, attnT_tiles.pop(s))
                                )
                        return finish_av_div, finish_transpose

                    for c in range(NCHUNK):
                        emit_qkt_exp(c)
                        if first_slot:
                            # pair-0 v groups trickle one step ahead of the
                            # AV group that consumes them, so AV never waits
                            # on the psum->v copies
                            v_group(0, 2 * c)
                            v_group(0, 2 * c + 1)
                        if c == 0:
                            if prev_finish is not None:
                                prev_finish[0]()
                        else:
                            emit_av(c - 1)
                            if c == 1 and prev_finish is not None:
                                prev_finish[1]()
                        if first_slot:
                            for f in forced.get(c, ()):
                                f()
                        else:
                            if filler:
                                npop = -(-len(filler) // max(pop_steps_left, 1))
                                if slot_idx == len(slots) - 1:
                                    # the final slot has no successor to
                                    # absorb leftovers but also starves on
                                    # PE: stretch the remaining fillers one
                                    # per step to the end
                                    npop = 1
                                for _ in range(min(npop, len(filler))):
                                    filler.pop(0)()
                            pop_steps_left -= 1

                    prev_finish = make_finish()

                prev_finish[0]()
                prev_finish[1]()
                for f in filler:
                    f()

            from contextlib import nullcontext
            with (tc.For_i(0, reps, 1) if reps else nullcontext()):
                body()

    nc.compile()
    return nc


def _host_prep(x, w_qkv, w_proj):
    bf16 = ml_dtypes.bfloat16
    in_maps = []
    for c in range(8):
        b, hg = c // 2, c % 2
        r0 = 384 * hg
        wq = w_qkv[r0 : r0 + 384] * SCALE
        wk = w_qkv[768 + r0 : 768 + r0 + 384]
        wvv = w_qkv[1536 + r0 : 1536 + r0 + 384]
        wqk = np.concatenate([wq, wk], axis=0)
        in_maps.append({
            "xt": np.ascontiguousarray(x[b].T).astype(bf16),
            "wqk": np.ascontiguousarray(wqk.T).astype(bf16),
            "wv": np.ascontiguousarray(wvv.T).astype(bf16),
            "wp": np.ascontiguousarray(w_proj[:, r0 : r0 + 384].T).astype(bf16),
        })
    return in_maps


def _get_fn():
    if "fn" in _CACHED:
        return _CACHED["fn"]

    import jax
    from jax.sharding import Mesh, PartitionSpec
    from jax.experimental.shard_map import shard_map
    from concourse import bass2jax
    from concourse.bass2jax import _bass_exec_p, install_neuronx_cc_hook

    install_neuronx_cc_hook()
    nc = build_core_program()

    in_names = ["xt", "wqk", "wv", "wp"]
    out_avals = [jax.core.ShapedArray((N, DIM), np.float32)]
    partition_name = nc.partition_id_tensor.name if nc.partition_id_tensor else None

    def _body(xt, wqk, wvv, wp, yzero):
        operands = [xt, wqk, wvv, wp, yzero]
        names = in_names + ["y"]
        if nc.dbg_addr is not None:
            operands.append(np.zeros((1, 2), np.uint32))
            names.append(nc.dbg_addr.name)
        if partition_name is not None:
            operands.append(bass2jax.partition_id_tensor())
            names.append(partition_name)
        outs = _bass_exec_p.bind(
            *operands,
            out_avals=tuple(out_avals),
            in_names=tuple(names),
            out_names=("y",),
            lowering_input_output_aliases=(),
            sim_require_finite=True,
            sim_require_nnan=True,
            nc=nc,
        )
        return outs[0]

    devices = jax.devices()[:8]
    mesh = Mesh(np.asarray(devices), ("core",))
    fn = jax.jit(
        shard_map(
            _body, mesh=mesh,
            in_specs=(PartitionSpec("core"),) * 5,
            out_specs=PartitionSpec("core"),
            check_rep=False,
        ),
        keep_unused=True,
    )
    _CACHED["fn"] = fn
    return fn


def _run(in_maps):
    import jax

    fn = _get_fn()
    concat_in = [
        np.concatenate([m[name] for m in in_maps], axis=0)
        for name in ["xt", "wqk", "wv", "wp"]
    ]
    yzero = np.zeros((8 * N, DIM), np.float32)
    out = jax.block_until_ready(fn(*concat_in, yzero))
    return np.asarray(out).reshape(8, N, DIM)


def kernel(x, w_qkv, w_proj, b_proj):
    x = np.asarray(x, dtype=np.float32)
    w_qkv = np.asarray(w_qkv, dtype=np.float32)
    w_proj = np.asarray(w_proj, dtype=np.float32)
    b_proj = np.asarray(b_proj, dtype=np.float32)

    in_maps = _host_prep(x, w_qkv, w_proj)
    parts = _run(in_maps)

    y = np.empty((B, N, DIM), dtype=np.float32)
    for b in range(B):
        y[b] = parts[2 * b] + parts[2 * b + 1] + b_proj
    return y


# revision 49
# speedup vs baseline: 1.0017x; 1.0017x over previous
"""Trainium2 Bass kernel for nn_Attention (B=4, N=2048, 12 heads, d=64).

Per-core work (core = (batch b, head-group hg)): 6 heads of attention over
N=2048, plus its slice of the qkv/out projections. Host splits w_qkv/w_proj
by head group, pre-scales q, and sums the two half-projections per batch
(plus bias) at the end. No collectives — every core's inputs are host-sliced.

Design notes (vs the V2 baseline, 355us HW / 258us CoreSim):
- The softmax exp stream on the ACT engine is the binding resource
  (~1 el/cycle/lane @1.2GHz + ~350cyc/instr overhead): everything is
  organized to keep it gapless. Score chunks [128kv x 1024q] ping-pong
  through 2x2 PSUM banks; exp instructions span the full chunk.
- A third of the h2=1 exp chunks run on DVE instead, via a Schraudolph
  fast-exp (int16(x*2^7/ln2 + 127*128-7) bitcast to bf16, ~1.8% rms);
  rebalances ACT vs the otherwise lightly-loaded DVE. Adds ~2e-3 rel err.
- AV runs in [q, d] orientation (lhsT = exp-score tile, rhs = v plus a
  ones-column for the softmax denominator), 65-wide streams: 2x fewer PE
  cycles than the [d, q] form under the cost model, wash on HW. The
  normalize is then a per-partition tensor_scalar; a PE transpose restores
  [d, q] for the projection lhsT.
- Minimal lead-in (2 qk groups on a packed lead DMA), remaining qkv/proj
  work trickles through the chunk steps as paced fillers; slot finishes are
  two-phase (AV+divide at next slot's step 0, transposes at step 1) so PE
  never waits on DVE; last strip's projections drain inline per q-tile.
- PE p-state warmup on the identity tile during the initial DMA wait.
"""

import sys

if "/opt/trn_rl_repo" not in sys.path:
    sys.path.insert(0, "/opt/trn_rl_repo")

import numpy as np
import ml_dtypes

import concourse.bacc as bacc
import concourse.mybir as mybir
import concourse.tile as tile
from concourse.masks import make_identity

FP32 = mybir.dt.float32
BF16 = mybir.dt.bfloat16
AF = mybir.ActivationFunctionType

DIM = 768
HEAD_DIM = 64
SCALE = HEAD_DIM ** -0.5
B, N = 4, 2048
HG = 6
CC = DIM // 128
PAIRS = HG // 2
S = N // 512
J = N // 128
CH = 2                      # kv blocks per score chunk
NCHUNK = J // CH            # 8 chunks per (head, strip)

# Schraudolph-style fast exp on DVE: bf16(x) ~ bitcast_bf16(int16(x*A + B))
# with A = 2^7/ln2 and B = 127*2^7 - C. C tuned numerically for min RMS
# relative error (~1.8%, max ~4%) under truncation. A third of the h2=1
# exp chunks run on DVE to offload the saturated ACT engine.
FAST_EXP_A = 128.0 / float(np.log(2.0))
FAST_EXP_B = 16256.0 - 7.0

_CACHED = {}


def build_core_program(reps=0, ablate=()):
    nc = bacc.Bacc("TRN2", debug=False, target_bir_lowering=False, num_devices=1)

    xt_d = nc.dram_tensor("xt", [DIM, N], BF16, kind="ExternalInput")
    wqk_d = nc.dram_tensor("wqk", [DIM, DIM], BF16, kind="ExternalInput")
    wv_d = nc.dram_tensor("wv", [DIM, HG * 64], BF16, kind="ExternalInput")
    wp_d = nc.dram_tensor("wp", [HG * 64, DIM], BF16, kind="ExternalInput")
    y_d = nc.dram_tensor("y", [N, DIM], FP32, kind="ExternalOutput")

    with tile.TileContext(nc) as tc:
        with (
            tc.tile_pool(name="persist", bufs=1) as persist,
            tc.tile_pool(name="exps", bufs=12) as exps_pool,
            tc.tile_pool(name="attnt", bufs=5) as attnt_pool,
            tc.tile_pool(name="small", bufs=6) as small_pool,
            tc.tile_pool(name="ysb", bufs=3) as y_pool,
            tc.tile_pool(name="ps_score", bufs=2, space="PSUM") as ps_score,
            tc.tile_pool(name="ps_av", bufs=2, space="PSUM") as ps_av,
            tc.tile_pool(name="ps_misc", bufs=2, space="PSUM") as ps_misc,
        ):
            xT = persist.tile([128, CC, N], BF16)
            wqk = persist.tile([128, CC, DIM], BF16)
            wv = persist.tile([128, CC, HG * 64], BF16)
            wp = persist.tile([128, PAIRS, DIM], BF16)
            qkT = persist.tile([128, CC, N], BF16)
            v = persist.tile([128, J, HG, 65], BF16)
            ident = persist.tile([128, 128], BF16)

            # wqk on SP and xT on Pool so the two issue streams run in
            # parallel (DMA issue is ~0.6us each and gates the lead-in);
            # wv/wp follow once the critical tensors are queued.
            xt_r = xt_d.ap().rearrange("(o p) n -> p o n", p=128)
            wqk_r = wqk_d.ap().rearrange("(o p) n -> p o n", p=128)
            wv_r = wv_d.ap().rearrange("(o p) n -> p o n", p=128)
            # The lead-in qk groups (pair 0, strip 0) only need wqk cols
            # {0:128, 384:512} and x strip 0: those land first as two packed
            # transfers (SP- and Pool-issued, in parallel); remainders and
            # wv/wp follow. Writers are disjoint so the lead-in reads never
            # wait on remainder DMAs.
            nc.sync.dma_start(out=wqk[:, :, 0:128], in_=wqk_r[:, :, 0:128])
            nc.scalar.dma_start(
                out=xT[:, 0:3, 0:512], in_=xt_r[:, 0:3, 0:512]
            )
            nc.scalar.dma_start(
                out=xT[:, 3:6, 0:512], in_=xt_r[:, 3:6, 0:512]
            )
            nc.sync.dma_start(
                out=wqk[:, :, 384:512], in_=wqk_r[:, :, 384:512]
            )
            for cc in range(CC):
                nc.sync.dma_start(out=wv[:, cc], in_=wv_r[:, cc])
            for cc in range(CC):
                nc.sync.dma_start(
                    out=xT[:, cc, 512:N], in_=xt_r[:, cc, 512:N]
                )
            nc.sync.dma_start(
                out=wqk[:, :, 128:384], in_=wqk_r[:, :, 128:384]
            )
            nc.sync.dma_start(
                out=wqk[:, :, 512:DIM], in_=wqk_r[:, :, 512:DIM]
            )
            nc.sync.dma_start(out=wp, in_=wp_d.ap().rearrange("(o p) n -> p o n", p=128))
            make_identity(nc, ident)
            # only the denominator ones-column needs initializing; the d
            # columns are fully written by v_groups
            nc.vector.memset(v[:, :, :, 64:65], 1.0)

            # warm the PE p-state during the DMA wait: a run of no-dep
            # matmuls on the identity tile carries the engine through the
            # slow ramp so the lead-in runs at full clock.
            # warmup lives in the AV pool: its first real allocation isn't
            # written until ~12us in, while a misc-pool slot would make the
            # second lead-in qk group wait for the whole warmup run
            warm = ps_av.tile([128, 4, 128], FP32, tag="av", name="warm")
            for _ in range(42):
                nc.tensor.matmul(warm[:, 0, 0:128], ident, ident,
                                 start=True, stop=True)

            # ---------- psum-group building blocks ----------

            def qk_group(ot, s):
                ps = ps_misc.tile([128, 512], FP32, tag="m")
                for cc in range(CC):
                    nc.tensor.matmul(
                        ps,
                        wqk[:, cc, ot * 128 : ot * 128 + 128],
                        xT[:, cc, s * 512 : s * 512 + 512],
                        start=(cc == 0), stop=(cc == CC - 1),
                    )
                nc.vector.tensor_copy(out=qkT[:, ot, s * 512 : s * 512 + 512], in_=ps)

            def v_group(p, nt):
                psv = ps_misc.tile([128, 512], FP32, tag="m")
                for cc in range(CC):
                    nc.tensor.matmul(
                        psv[:, 0:128],
                        xT[:, cc, nt * 128 : nt * 128 + 128],
                        wv[:, cc, p * 128 : p * 128 + 128],
                        start=(cc == 0), stop=(cc == CC - 1),
                    )
                for h2 in range(2):
                    nc.vector.tensor_copy(
                        out=v[:, nt, 2 * p + h2, 0:64],
                        in_=psv[:, h2 * 64 : h2 * 64 + 64],
                    )

            def proj_group(s, attnT, nt, og, ow, ysb):
                psy = ps_misc.tile([128, 512], FP32, tag="m")
                for cc in range(PAIRS):
                    nc.tensor.matmul(
                        psy[:, 0:ow],
                        attnT[:, cc, nt * 128 : nt * 128 + 128],
                        wp[:, cc, og : og + ow],
                        start=(cc == 0), stop=(cc == PAIRS - 1),
                    )
                nc.vector.tensor_copy(out=ysb[:, og : og + ow], in_=psy[:, 0:ow])
                # ship each column half as soon as it's ready so the final
                # tile's DMA only covers 256 columns; first halves issue from
                # the idle Pool queue so the drain's issues don't serialize
                # on SP
                row = s * 512 + nt * 128
                eng = nc.gpsimd if og == 0 else nc.sync
                eng.dma_start(
                    out=y_d.ap()[row : row + 128, og : og + ow],
                    in_=ysb[:, og : og + ow],
                )

            def qkv_pair_fillers(p):
                out = []
                for ot in (p, PAIRS + p):
                    for s in range(S):
                        out.append(lambda ot=ot, s=s: qk_group(ot, s))
                for nt in range(J):
                    out.append(lambda nt=nt: v_group(p, nt))
                return out

            def proj_fillers(s, attnT):
                out = []
                for nt in range(4):
                    ysb = y_pool.tile([128, DIM], FP32, tag="y")
                    for og, ow in ((0, 512), (512, 256)):
                        out.append(
                            lambda nt=nt, og=og, ow=ow, ysb=ysb:
                                proj_group(s, attnT, nt, og, ow, ysb)
                        )
                return out

            def body():
                # minimal lead-in: q strip 0 and k strip 0 of pair 0 only
                qk_group(0, 0)
                qk_group(PAIRS, 0)

                # the other 6 lead-in groups, forced into slot (0,0) steps
                # just before the score chunks that need them:
                # k strip s' is read by chunks 2s'..2s'+1; q strip s is read
                # from slot (0,s) on.
                forced = {
                    1: [lambda: qk_group(PAIRS, 1)],
                    2: [lambda: qk_group(0, 1)],
                    3: [lambda: qk_group(PAIRS, 2)],
                    4: [lambda: qk_group(0, 2)],
                    5: [lambda: qk_group(PAIRS, 3)],
                    6: [lambda: qk_group(0, 3)],
                }

                filler = []
                attnT_tiles = {}
                gchunk = [0]  # global chunk counter for fast-exp selection
                slots = [(hp, s) for hp in range(PAIRS) for s in range(S)]
                # filler pop opportunities remaining (every step of each
                # non-first slot)
                pop_steps_left = (len(slots) - 1) * NCHUNK
                prev_finish = None

                for slot_idx, (hp, s) in enumerate(slots):
                    first_slot = slot_idx == 0
                    if s == 0 and hp + 1 < PAIRS:
                        filler.extend(qkv_pair_fillers(hp + 1))
                    if hp == 0:
                        at = attnt_pool.tile(
                            [128, PAIRS, 512], BF16, tag="attnT",
                            name=f"attnT{s}",
                        )
                        attnT_tiles[s] = at
                    attnT = attnT_tiles[s]

                    pav = {}
                    for h2 in range(2):
                        pav[h2] = ps_av.tile(
                            [128, 4, 128], FP32, tag="av", name=f"pav{h2}"
                        )

                    expS = {0: [None] * NCHUNK, 1: [None] * NCHUNK}

                    def emit_qkt_exp(c, s=s, hp=hp, expS=expS):
                        g = gchunk[0]
                        gchunk[0] += 1
                        # which h2 (if any) of this chunk runs fast-exp on DVE
                        fast_h2 = ((g // 3) % 2) if g % 3 == 1 else -1
                        for h2, base in ((0, 0), (1, 64)):
                            pss = ps_score.tile(
                                [128, 512 * CH], FP32, tag="sc"
                            )
                            for jj in range(CH):
                                j = c * CH + jj
                                if "qkt" in ablate:
                                    continue
                                nc.tensor.matmul(
                                    pss[:, jj * 512 : jj * 512 + 512],
                                    qkT[base : base + 64, PAIRS + hp,
                                        j * 128 : j * 128 + 128],
                                    qkT[base : base + 64, hp,
                                        s * 512 : s * 512 + 512],
                                    start=True, stop=True,
                                    tile_position=(base, 0),
                                )
                            et = exps_pool.tile(
                                [128, 512 * CH], BF16, tag="e"
                            )
                            if "exp" in ablate:
                                nc.vector.memset(et[:, 0:4], 1.0)
                            elif h2 == fast_h2:
                                nc.vector.tensor_scalar(
                                    et.bitcast(mybir.dt.int16),
                                    pss,
                                    FAST_EXP_A,
                                    FAST_EXP_B,
                                    mybir.AluOpType.mult,
                                    mybir.AluOpType.add,
                                )
                            else:
                                nc.scalar.activation(
                                    out=et, in_=pss, func=AF.Exp,
                                )
                            expS[h2][c] = et

                    def emit_av(c, hp=hp, pav=pav, expS=expS):
                        # AV in [q, d] orientation: out[128 q-tile, 65] per
                        # (head, q-tile), accumulated over the 16 kv blocks.
                        # lhsT = exp-score slice [128 kv, 128 q]; rhs =
                        # v[kv, d + ones-col] gives the softmax denominator
                        # in column 64.
                        if "av" in ablate:
                            return
                        for h2 in range(2):
                            h = 2 * hp + h2
                            et = expS[h2][c]
                            for jj in range(CH):
                                j = c * CH + jj
                                for qt in range(4):
                                    # start/stop once per pav BANK: start
                                    # marks the whole 2KB zero-region
                                    # pending-zero, so the other qt groups'
                                    # first writes land on zeroed bytes.
                                    nc.tensor.matmul(
                                        pav[h2][:, qt, 0:65],
                                        et[:, jj * 512 + qt * 128
                                           : jj * 512 + qt * 128 + 128],
                                        v[:, j, h, :],
                                        start=(j == 0 and qt == 0),
                                        stop=(j == J - 1 and qt == 3),
                                        skip_group_check=(not (j == 0 and qt == 0)
                                                          and not (j == J - 1 and qt == 3)),
                                    )

                    last_slot = slot_idx == len(slots) - 1

                    def make_finish(hp=hp, s=s, pav=pav, expS=expS,
                                    attnT=attnT, first_slot=first_slot,
                                    last_slot=last_slot, emit_av=emit_av):
                        # two-phase finish: phase 1 (next slot's step 0)
                        # emits the last AV group + the DVE normalize chain
                        # (divide rows by the ones-column denominator, a
                        # per-partition scalar); phase 2 (next slot's step 1)
                        # emits the PE transposes back to [d, q] layout, by
                        # which time DVE has produced the normalized tiles.
                        prs = []

                        def emit_div(qt):
                            pr = small_pool.tile([128, 128], BF16, tag="p")
                            prs.append(pr)
                            for h2 in range(2):
                                rc = small_pool.tile([128, 1], FP32, tag="r")
                                if "av" in ablate:
                                    nc.vector.memset(rc, 1.0)
                                else:
                                    nc.vector.reciprocal(
                                        out=rc, in_=pav[h2][:, qt, 64:65]
                                    )
                                nc.vector.tensor_scalar(
                                    pr[:, h2 * 64 : h2 * 64 + 64],
                                    pav[h2][:, qt, 0:64],
                                    rc,
                                    None,
                                    mybir.AluOpType.mult,
                                )

                        def emit_transpose(qt):
                            pt = ps_misc.tile([128, 128], BF16, tag="m")
                            nc.tensor.transpose(pt, prs[qt], ident)
                            nc.vector.tensor_copy(
                                out=attnT[:, hp, qt * 128 : qt * 128 + 128],
                                in_=pt,
                            )

                        def finish_av_div():
                            emit_av(NCHUNK - 1)
                            if not last_slot:
                                for qt in range(4):
                                    emit_div(qt)

                        def finish_transpose():
                            if last_slot:
                                # drain: interleave per q-tile so each
                                # projection only waits on its own tile's
                                # divide/transpose/copyback chain
                                last_projs = proj_fillers(
                                    s, attnT_tiles.pop(s)
                                )
                                for qt in range(4):
                                    emit_div(qt)
                                    emit_transpose(qt)
                                    last_projs.pop(0)()
                                    last_projs.pop(0)()
                                return
                            for qt in range(4):
                                emit_transpose(qt)
                            if hp == PAIRS - 1:
                                filler.extend(
                                    proj_fillers(s, attnT_tiles.pop(s))
                                )
                        return finish_av_div, finish_transpose

                    for c in range(NCHUNK):
                        emit_qkt_exp(c)
                        if first_slot:
                            # pair-0 v groups trickle one step ahead of the
                            # AV group that consumes them, so AV never waits
                            # on the psum->v copies
                            v_group(0, 2 * c)
                            v_group(0, 2 * c + 1)
                        if c == 0:
                            if prev_finish is not None:
                                prev_finish[0]()
                        else:
                            emit_av(c - 1)
                            if c == 1 and prev_finish is not None:
                                prev_finish[1]()
                        if first_slot:
                            for f in forced.get(c, ()):
                                f()
                        else:
                            if filler:
                                npop = -(-len(filler) // max(pop_steps_left, 1))
                                if slot_idx == len(slots) - 1:
                                    # the final slot has no successor to
                                    # absorb leftovers but also starves on
                                    # PE: stretch the remaining fillers one
                                    # per step to the end
                                    npop = 1
                                for _ in range(min(npop, len(filler))):
                                    filler.pop(0)()
                            pop_steps_left -= 1

                    prev_finish = make_finish()

                prev_finish[0]()
                prev_finish[1]()
                for f in filler:
                    f()

            from contextlib import nullcontext
            with (tc.For_i(0, reps, 1) if reps else nullcontext()):
                body()

    nc.compile()
    return nc


def _host_prep(x, w_qkv, w_proj):
    bf16 = ml_dtypes.bfloat16
    in_maps = []
    for c in range(8):
        b, hg = c // 2, c % 2
        r0 = 384 * hg
        wq = w_qkv[r0 : r0 + 384] * SCALE
        wk = w_qkv[768 + r0 : 768 + r0 + 384]
        wvv = w_qkv[1536 + r0 : 1536 + r0 + 384]
        wqk = np.concatenate([wq, wk], axis=0)
        in_maps.append({
            "xt": np.ascontiguousarray(x[b].T).astype(bf16),
            "wqk": np.ascontiguousarray(wqk.T).astype(bf16),
            "wv": np.ascontiguousarray(wvv.T).astype(bf16),
            "wp": np.ascontiguousarray(w_proj[:, r0 : r0 + 384].T).astype(bf16),
        })
    return in_maps


def _get_fn():
    if "fn" in _CACHED:
        return _CACHED["fn"]

    import jax
    from jax.sharding import Mesh, PartitionSpec
    from jax.experimental.shard_map import shard_map
    from concourse import bass2jax
    from concourse.bass2jax import _bass_exec_p, install_neuronx_cc_hook

    install_neuronx_cc_hook()
    nc = build_core_program()

    in_names = ["xt", "wqk", "wv", "wp"]
    out_avals = [jax.core.ShapedArray((N, DIM), np.float32)]
    partition_name = nc.partition_id_tensor.name if nc.partition_id_tensor else None

    def _body(xt, wqk, wvv, wp, yzero):
        operands = [xt, wqk, wvv, wp, yzero]
        names = in_names + ["y"]
        if nc.dbg_addr is not None:
            operands.append(np.zeros((1, 2), np.uint32))
            names.append(nc.dbg_addr.name)
        if partition_name is not None:
            operands.append(bass2jax.partition_id_tensor())
            names.append(partition_name)
        outs = _bass_exec_p.bind(
            *operands,
            out_avals=tuple(out_avals),
            in_names=tuple(names),
            out_names=("y",),
            lowering_input_output_aliases=(),
            sim_require_finite=True,
            sim_require_nnan=True,
            nc=nc,
        )
        return outs[0]

    devices = jax.devices()[:8]
    mesh = Mesh(np.asarray(devices), ("core",))
    fn = jax.jit(
        shard_map(
            _body, mesh=mesh,
            in_specs=(PartitionSpec("core"),) * 5,
            out_specs=PartitionSpec("core"),
            check_rep=False,
        ),
        keep_unused=True,
    )
    _CACHED["fn"] = fn
    return fn


def _run(in_maps):
    import jax

    fn = _get_fn()
    concat_in = [
        np.concatenate([m[name] for m in in_maps], axis=0)
        for name in ["xt", "wqk", "wv", "wp"]
    ]
    yzero = np.zeros((8 * N, DIM), np.float32)
    out = jax.block_until_ready(fn(*concat_in, yzero))
    return np.asarray(out).reshape(8, N, DIM)


def kernel(x, w_qkv, w_proj, b_proj):
    x = np.asarray(x, dtype=np.float32)
    w_qkv = np.asarray(w_qkv, dtype=np.float32)
    w_proj = np.asarray(w_proj, dtype=np.float32)
    b_proj = np.asarray(b_proj, dtype=np.float32)

    in_maps = _host_prep(x, w_qkv, w_proj)
    parts = _run(in_maps)

    y = np.empty((B, N, DIM), dtype=np.float32)
    for b in range(B):
        y[b] = parts[2 * b] + parts[2 * b + 1] + b_proj
    return y
